# revision 1
# baseline (speedup 1.0000x reference)
"""Trainium2 Bass kernel for the ExemplarHead classification problem.

Math: per (task, way), with R the 5x1024 class reps (support+noise),
H = I - (1/5)11^T the centering matrix, G = H R R^T H (5x5 Gram of the
centered reps), the SVD-based projection head reduces exactly to

    C   = W R,  W = I - lam * (lam I + G)^{-1} H          (5x5 per block)
    logits[q, (w,s)] = (2 q.C_(w,s) - ||q||^2 - ||C_(w,s)||^2) / d

(lam I + G) has kappa <= 1.25, so its inverse is computed with a scaled
Newton iteration (Y1 = 2I - aK, two quadratic steps; final rel err ~6e-7).
All 20 (task,way) blocks per core are handled as one masked block-diagonal
100x100 problem.

Sharding: data-parallel over the 32 tasks -> 4 tasks per NeuronCore x 8.
"""

import numpy as np

import concourse.bass as bass
import concourse.mybir as mybir
import concourse.tile as tile
from concourse import bacc
from concourse.bass_utils import run_bass_kernel_spmd

F32 = mybir.dt.float32
AF = mybir.ActivationFunctionType
ALU = mybir.AluOpType

LAM = 100000.0
GMAX_BOUND = 40000.0            # safe bound on ||G|| (observed max ~2.2e4)
ALPHA = 2.0 / (2.0 * LAM + GMAX_BOUND)

N_CORES = 8
T_FULL, NQ, D = 32, 75, 1024
NW, NS = 5, 5
TPC = T_FULL // N_CORES          # tasks per core = 4
NR = TPC * NW * NS               # R rows per core = 100
NCH = D // 128                   # 8 contraction chunks
NJ = NW * NS                     # 25 (way,shot) pairs per task
CP_COLS = 805                    # packed constant tile columns


def _host_consts():
    """One packed constant tensor [128, 805] (single DMA -> single sem lane).

    cols 0:600   six 100x100 matrices (rows 0..99)
    cols 600:728 128x128 identity
    col  728     -0.5 column
    cols 729:804 row of ones on partition 0
    """
    H5 = np.eye(NS) - np.ones((NS, NS)) / NS
    H_bd = np.kron(np.eye(TPC * NW), H5).astype(np.float32)       # [100,100]
    blockmask = np.kron(np.eye(TPC * NW), np.ones((NS, NS))).astype(np.float32)
    eye = np.eye(NR, dtype=np.float32)
    mats = [
        H_bd,                                   # 0: H (centering, block diag)
        (ALPHA * blockmask).astype(np.float32),  # 1: alpha * mask
        (ALPHA * LAM * eye).astype(np.float32),  # 2: alpha*lam*I
        (2.0 * eye).astype(np.float32),          # 3: 2I
        eye,                                     # 4: I
        (ALPHA * LAM * H_bd).astype(np.float32),  # 5: alpha*lam*H
    ]
    cP = np.zeros((128, CP_COLS), dtype=np.float32)
    for m, mat in enumerate(mats):
        cP[0:NR, m * NR:(m + 1) * NR] = mat
    cP[:, 600:728] = np.eye(128, dtype=np.float32)
    cP[:, 728] = -0.5
    cP[0, 729:804] = 1.0
    return cP


def build_nc():
    nc = bacc.Bacc("TRN2")

    q_d = nc.declare_dram_parameter("q", [TPC, NQ, D], F32, isOutput=False)
    sn_d = nc.declare_dram_parameter("sn", [NR, D], F32, isOutput=False)
    nz_d = nc.declare_dram_parameter("nz", [NR, D], F32, isOutput=False)
    cP_d = nc.declare_dram_parameter("cP", [128, CP_COLS], F32, isOutput=False)
    out_d = nc.declare_dram_parameter("out", [TPC, NQ, NJ], F32, isOutput=True)

    with tile.TileContext(nc) as tc:
        with (
            tc.tile_pool(name="consts", bufs=1) as consts,
            tc.tile_pool(name="sb", bufs=1) as sb,
            tc.tile_pool(name="scr", bufs=2) as scr,
            tc.tile_pool(name="pipe", bufs=3, space="PSUM") as pipe,
            tc.tile_pool(name="gp", bufs=1, space="PSUM") as gp,
            tc.tile_pool(name="cnp", bufs=1, space="PSUM") as cnp,
            tc.tile_pool(name="qcp", bufs=2, space="PSUM") as qcp,
        ):
            # ---- constants: ONE DMA on the HWDGE ring (first in FIFO) ----
            cP = consts.tile([128, CP_COLS], F32)
            nc.sync.dma_start(out=cP, in_=cP_d[:])
            c_H = cP[0:NR, 0:NR]
            c_amask = cP[0:NR, NR:2 * NR]
            c_alI = cP[0:NR, 2 * NR:3 * NR]
            c_2I = cP[0:NR, 3 * NR:4 * NR]
            c_I = cP[0:NR, 4 * NR:5 * NR]
            c_alH = cP[0:NR, 5 * NR:6 * NR]
            ident = cP[:, 600:728]
            negh = cP[:, 728:729]
            ones75 = cP[0:1, 729:729 + NQ]

            # early DVE touch of cP so later DVE ops don't re-wait its sem
            warm = sb.tile([1, 1], F32)
            nc.vector.tensor_copy(warm, cP[0:1, 0:1])

            # ---- R = support + noise via SWDGE accumulate-DMA ----
            r_sb = sb.tile([NR, D], F32)
            HD = D // 2
            for h in range(2):
                sl = slice(h * HD, (h + 1) * HD)
                nc.gpsimd.dma_start(out=r_sb[:, sl], in_=sn_d[:, sl])
                nc.gpsimd.dma_start(out=r_sb[:, sl], in_=nz_d[:, sl],
                                    accum_op=ALU.add)

            # ---- query loads on the HWDGE ring after cP ----
            q_sb = sb.tile([NQ, TPC * D], F32)
            for t in range(TPC):
                nc.sync.dma_start(out=q_sb[:, t * D:(t + 1) * D], in_=q_d[t])

            # ---- RcT = (H R)^T by chunks: psum[128,100] = R_chunk^T @ H ----
            rct_sb = sb.tile([128, NCH * NR], F32)
            for p in range(2):
                rct_ps = pipe.tile([128, 4 * NR], F32, space="PSUM", tag="pp")
                for kk in range(4):
                    k = 4 * p + kk
                    nc.tensor.matmul(rct_ps[:, kk * NR:(kk + 1) * NR],
                                     lhsT=r_sb[:, k * 128:(k + 1) * 128],
                                     rhs=c_H, start=True, stop=True)
                nc.vector.tensor_copy(rct_sb[:, p * 4 * NR:(p + 1) * 4 * NR], rct_ps)

            # ---- G = sum_k RcT_k^T RcT_k  (= H R R^T H) ----
            g_ps = gp.tile([NR, NR], F32, space="PSUM")
            for k in range(NCH):
                rct_k = rct_sb[:, k * NR:(k + 1) * NR]
                nc.tensor.matmul(g_ps, lhsT=rct_k, rhs=rct_k,
                                 start=(k == 0), stop=(k == NCH - 1))

            # ---- K_alpha = alpha*(mask o G) + alpha*lam*I ; Newton inverse ----
            gm_sb = sb.tile([NR, NR], F32)
            nc.vector.tensor_mul(gm_sb, g_ps, c_amask)
            ka_sb = sb.tile([NR, NR], F32)
            nc.vector.tensor_add(ka_sb, gm_sb, c_alI)
            y_sb = sb.tile([NR, NR], F32)
            nc.vector.tensor_sub(y_sb, c_2I, ka_sb)        # Y1 = 2I - Ka
            for it in range(2):
                p_ps = pipe.tile([NR, NR], F32, space="PSUM", tag="pp")
                nc.tensor.matmul(p_ps, lhsT=ka_sb, rhs=y_sb, start=True, stop=True)
                qq_sb = sb.tile([NR, NR], F32, tag="qqn", name=f"qq{it}")
                nc.vector.tensor_sub(qq_sb, c_2I, p_ps)    # 2I - Ka Y
                yn_ps = pipe.tile([NR, NR], F32, space="PSUM", tag="pp")
                nc.tensor.matmul(yn_ps, lhsT=y_sb, rhs=qq_sb, start=True, stop=True)
                y2_sb = sb.tile([NR, NR], F32, tag="ynn", name=f"yn{it}")
                nc.scalar.copy(y2_sb, yn_ps)
                y_sb = y2_sb

            # ---- W^T = I - (alpha*lam*H) Y ----
            hy_ps = pipe.tile([NR, NR], F32, space="PSUM", tag="pp")
            nc.tensor.matmul(hy_ps, lhsT=c_alH, rhs=y_sb, start=True, stop=True)
            wt_sb = sb.tile([NR, NR], F32)
            nc.vector.tensor_sub(wt_sb, c_I, hy_ps)

            # ---- C^T chunks = R_chunk^T @ W^T ; squares for ||C||^2 ----
            ct_sb = sb.tile([128, NCH * NR], F32)
            csq_sb = sb.tile([128, NCH * NR], F32)
            for p in range(2):
                ct_ps = pipe.tile([128, 4 * NR], F32, space="PSUM", tag="pp")
                for kk in range(4):
                    k = 4 * p + kk
                    nc.tensor.matmul(ct_ps[:, kk * NR:(kk + 1) * NR],
                                     lhsT=r_sb[:, k * 128:(k + 1) * 128],
                                     rhs=wt_sb, start=True, stop=True)
                sl = slice(p * 4 * NR, (p + 1) * 4 * NR)
                nc.vector.tensor_copy(ct_sb[:, sl], ct_ps)
                nc.scalar.activation(csq_sb[:, sl], ct_ps, AF.Square)

            # ---- cn row: [1,100] = sum_d -0.5 * C^T(d,j)^2 ----
            cn_ps = cnp.tile([1, NR], F32, space="PSUM")
            for k in range(NCH):
                nc.tensor.matmul(cn_ps, lhsT=negh,
                                 rhs=csq_sb[:, k * NR:(k + 1) * NR],
                                 start=(k == 0), stop=(k == NCH - 1))
            cn_sb = sb.tile([1, NR], F32)
            nc.scalar.copy(cn_sb, cn_ps)

            # ---- per-task: q^T via PE transpose, ||q||^2, QC, epilogue ----
            qt_sb = sb.tile([128, TPC * NCH * NQ], F32)   # [128, 4*8*75]
            qnorm = sb.tile([NQ, TPC], F32)
            qbias = sb.tile([NQ, TPC], F32)
            out_sb = sb.tile([NQ, TPC * NJ], F32)
            for t in range(TPC):
                qn_t = q_sb[:, t * D:(t + 1) * D]
                # ||q||^2 via ACT square + free-dim accumulate (1 DMA wait)
                sq_scr = scr.tile([NQ, D], F32, tag="sq")
                nc.scalar.activation(sq_scr, qn_t, AF.Square,
                                     accum_out=qnorm[:, t:t + 1])
                # qbias = -qn/D  (same-engine chain, no new cross-engine wait)
                nc.scalar.activation(qbias[:, t:t + 1], qnorm[:, t:t + 1],
                                     AF.Copy, scale=-1.0 / D)
                # transpose q_t by 128-chunks (packs of 4 -> one PSUM bank)
                for p in range(2):
                    qt_ps = pipe.tile([128, 4 * NQ], F32, space="PSUM", tag="pp")
                    for kk in range(4):
                        k = 4 * p + kk
                        nc.tensor.transpose(qt_ps[:, kk * NQ:(kk + 1) * NQ],
                                            qn_t[:, k * 128:(k + 1) * 128],
                                            ident[0:NQ, 0:NQ])
                    dst = qt_sb[:, (t * 8 + p * 4) * NQ:(t * 8 + p * 4 + 4) * NQ]
                    if p == 0:
                        nc.vector.tensor_copy(dst, qt_ps)
                    else:
                        nc.scalar.copy(dst, qt_ps)
                # QC accumulation: 8 chunks + cn-row augmentation
                qc_ps = qcp.tile([NQ, NJ], F32, space="PSUM", tag="qc",
                                 name=f"qc{t}")
                for k in range(NCH):
                    lhs = qt_sb[:, (t * 8 + k) * NQ:(t * 8 + k + 1) * NQ]
                    rhs = ct_sb[:, k * NR + t * NJ:k * NR + t * NJ + NJ]
                    nc.tensor.matmul(qc_ps, lhsT=lhs, rhs=rhs,
                                     start=(k == 0), stop=False)
                nc.tensor.matmul(qc_ps, lhsT=ones75,
                                 rhs=cn_sb[0:1, t * NJ:(t + 1) * NJ],
                                 start=False, stop=True)
                # logits = (2/D)*psum + (-qn/D): two 1-wait DVE ops
                tmp_t = scr.tile([NQ, NJ], F32, tag="ep")
                nc.vector.tensor_scalar_mul(tmp_t, qc_ps, 2.0 / D)
                nc.vector.tensor_scalar_add(out_sb[:, t * NJ:(t + 1) * NJ],
                                            tmp_t, qbias[:, t:t + 1])
                nc.sync.dma_start(out=out_d[t], in_=out_sb[:, t * NJ:(t + 1) * NJ])

    nc.finalize()
    return nc


_NC_CACHE = None


def _get_nc():
    global _NC_CACHE
    if _NC_CACHE is None:
        _NC_CACHE = build_nc()
    return _NC_CACHE


def make_in_maps(query, support, noise):
    query = np.asarray(query, dtype=np.float32)
    support = np.asarray(support, dtype=np.float32)
    noise = np.asarray(noise, dtype=np.float32)
    cP = _host_consts()
    in_maps = []
    for c in range(N_CORES):
        ts = slice(c * TPC, (c + 1) * TPC)
        in_maps.append({
            "q": np.ascontiguousarray(query[ts]),
            "sn": np.ascontiguousarray(support[ts]).reshape(NR, D),
            "nz": np.ascontiguousarray(
                noise[:, ts].transpose(1, 0, 2, 3)).reshape(NR, D),
            "cP": cP,
        })
    return in_maps


def kernel(query, support, noise, support_labels=None, n_way=None, n_shot=None,
           **_unused):
    nc = _get_nc()
    in_maps = make_in_maps(query, support, noise)
    res = run_bass_kernel_spmd(nc, in_maps, list(range(N_CORES)))
    outs = [np.asarray(r["out"]).reshape(TPC, NQ, NJ) for r in res.results]
    full = np.concatenate(outs, axis=0)            # (32, 75, 25)
    return full.reshape(T_FULL, NQ, NW, NS).astype(np.float32)



# revision 2
# speedup vs baseline: 1.6216x; 1.6216x over previous
"""Trainium2 Bass kernel for the ExemplarHead classification problem (v2, bf16).

Math: per (task, way), with R the 5x1024 class reps (support+noise),
H = I - (1/5)11^T, G = H R R^T H, the SVD head reduces exactly to
    C = W R,  W = I - lam * (lam I + G)^{-1} H
    logits[q,(w,s)] = (2 q.C - ||q||^2 - ||C||^2) / d
(lam I + G) inverse via one scaled Newton step (residual (I-aK)^4 ~ 8e-4,
below the bf16 noise floor). All 20 (task,way) blocks per core are one
masked block-diagonal 100x100 problem.

v2 changes vs v1 (60.4us):
 - all large matmuls in bf16 (1 PE cycle/col vs fp32's 2x2), psum fp32
 - q arrives pre-transposed from host (qT) -> no PE transposes at all
 - one Newton iteration instead of two
 - single packed output DMA; norm folds stay fp32 for accuracy

Sharding: data-parallel over the 32 tasks -> 4 tasks per NeuronCore x 8.
"""

import numpy as np
import ml_dtypes

import concourse.bass as bass
import concourse.mybir as mybir
import concourse.tile as tile
from concourse import bacc
from concourse.bass_utils import run_bass_kernel_spmd

F32 = mybir.dt.float32
BF16 = mybir.dt.bfloat16
AF = mybir.ActivationFunctionType
ALU = mybir.AluOpType

LAM = 100000.0
GMAX_BOUND = 40000.0            # safe bound on ||G|| (observed max ~2.2e4)
ALPHA = 2.0 / (2.0 * LAM + GMAX_BOUND)

N_CORES = 8
T_FULL, NQ, D = 32, 75, 1024
NW, NS = 5, 5
TPC = T_FULL // N_CORES          # tasks per core = 4
NR = TPC * NW * NS               # R rows per core = 100
NCH = D // 128                   # 8 contraction chunks
NJ = NW * NS                     # 25 (way,shot) pairs per task
CF_COLS = 475                    # fp32 const tile columns
CB_COLS = 201                    # bf16 const tile columns


def _host_consts():
    """Packed constant tiles. cF fp32 [128,475], cB bf16 [128,201].

    cF cols: 0:100 alpha*blockmask, 100:200 alpha*lam*I, 200:300 2I,
             300:400 I, col 400:475 ones row (partition 0)
    cB cols: 0:100 H (block-diag), 100:200 alpha*lam*H, col 200 = -0.5
    """
    H5 = np.eye(NS) - np.ones((NS, NS)) / NS
    H_bd = np.kron(np.eye(TPC * NW), H5).astype(np.float32)       # [100,100]
    blockmask = np.kron(np.eye(TPC * NW), np.ones((NS, NS))).astype(np.float32)
    eye = np.eye(NR, dtype=np.float32)
    cF = np.zeros((128, CF_COLS), dtype=np.float32)
    cF[0:NR, 0:NR] = ALPHA * blockmask
    cF[0:NR, NR:2 * NR] = ALPHA * LAM * eye
    cF[0:NR, 2 * NR:3 * NR] = 2.0 * eye
    cF[0:NR, 3 * NR:4 * NR] = eye
    cF[0, 4 * NR:4 * NR + NQ] = 1.0
    cB = np.zeros((128, CB_COLS), dtype=np.float32)
    cB[0:NR, 0:NR] = H_bd
    cB[0:NR, NR:2 * NR] = ALPHA * LAM * H_bd
    cB[:, 200] = -0.5
    return cF, cB.astype(ml_dtypes.bfloat16)


def build_nc():
    nc = bacc.Bacc("TRN2")

    qt_d = nc.declare_dram_parameter("qt", [NCH, 128, TPC * NQ], BF16,
                                     isOutput=False)
    qn_d = nc.declare_dram_parameter("qn", [NQ, TPC * D], BF16, isOutput=False)
    sn_d = nc.declare_dram_parameter("sn", [NR, D], F32, isOutput=False)
    nz_d = nc.declare_dram_parameter("nz", [NR, D], F32, isOutput=False)
    cF_d = nc.declare_dram_parameter("cF", [128, CF_COLS], F32, isOutput=False)
    cB_d = nc.declare_dram_parameter("cB", [128, CB_COLS], BF16, isOutput=False)
    out_d = nc.declare_dram_parameter("out", [NQ, TPC * NJ], F32, isOutput=True)

    with tile.TileContext(nc) as tc:
        with (
            tc.tile_pool(name="consts", bufs=1) as consts,
            tc.tile_pool(name="sb", bufs=1) as sb,
            tc.tile_pool(name="scr", bufs=2) as scr,
            tc.tile_pool(name="pipe", bufs=3, space="PSUM") as pipe,
            tc.tile_pool(name="gp", bufs=1, space="PSUM") as gp,
            tc.tile_pool(name="cnp", bufs=1, space="PSUM") as cnp,
            tc.tile_pool(name="qcp", bufs=2, space="PSUM") as qcp,
        ):
            # ---- constants first in the HWDGE FIFO ----
            cF = consts.tile([128, CF_COLS], F32)
            nc.sync.dma_start(out=cF, in_=cF_d[:])
            cB = consts.tile([128, CB_COLS], BF16)
            nc.sync.dma_start(out=cB, in_=cB_d[:])
            c_amask = cF[0:NR, 0:NR]
            c_alI = cF[0:NR, NR:2 * NR]
            c_2I = cF[0:NR, 2 * NR:3 * NR]
            c_I = cF[0:NR, 3 * NR:4 * NR]
            ones75 = cF[0:1, 4 * NR:4 * NR + NQ]
            c_Hb = cB[0:NR, 0:NR]
            c_alHb = cB[0:NR, NR:2 * NR]
            neghb = cB[:, 200:201]

            # early DVE touch so later DVE ops don't re-wait the const sems
            warm = sb.tile([1, 2], F32)
            nc.vector.tensor_copy(warm[0:1, 0:1], cF[0:1, 0:1])
            nc.vector.tensor_copy(warm[0:1, 1:2], cB[0:1, 0:1])

            # ---- q norms input + transposed q on HWDGE ring ----
            qn_nat = sb.tile([NQ, TPC * D], BF16)
            nc.sync.dma_start(out=qn_nat, in_=qn_d[:])
            qtb = sb.tile([128, NCH * TPC * NQ], BF16)
            for k in range(NCH):
                nc.sync.dma_start(out=qtb[:, k * 300:(k + 1) * 300],
                                  in_=qt_d[k])

            # ---- R = support + noise via SWDGE accumulate-DMA ----
            r_sb = sb.tile([NR, D], F32)
            HD = D // 2
            for h in range(2):
                sl = slice(h * HD, (h + 1) * HD)
                nc.gpsimd.dma_start(out=r_sb[:, sl], in_=sn_d[:, sl])
                nc.gpsimd.dma_start(out=r_sb[:, sl], in_=nz_d[:, sl],
                                    accum_op=ALU.add)
            rb = sb.tile([NR, D], BF16)
            for h in range(2):
                sl = slice(h * HD, (h + 1) * HD)
                nc.vector.tensor_copy(rb[:, sl], r_sb[:, sl])

            # ---- ||q||^2 per task (scalar engine, overlaps PE phase) ----
            qnorm = sb.tile([NQ, TPC], F32)
            qbias = sb.tile([NQ, TPC], F32)
            for t in range(TPC):
                sq_scr = scr.tile([NQ, D], BF16, tag="sq")
                nc.scalar.activation(sq_scr, qn_nat[:, t * D:(t + 1) * D],
                                     AF.Square, accum_out=qnorm[:, t:t + 1])
            nc.scalar.activation(qbias, qnorm, AF.Copy, scale=-1.0 / D)

            # ---- RcT = (H R)^T by chunks (bf16) ----
            rctb = sb.tile([128, NCH * NR], BF16)
            for p in range(2):
                rct_ps = pipe.tile([128, 4 * NR], F32, space="PSUM", tag="pp")
                for kk in range(4):
                    k = 4 * p + kk
                    nc.tensor.matmul(rct_ps[:, kk * NR:(kk + 1) * NR],
                                     lhsT=rb[:, k * 128:(k + 1) * 128],
                                     rhs=c_Hb, start=True, stop=True)
                nc.vector.tensor_copy(rctb[:, p * 4 * NR:(p + 1) * 4 * NR],
                                      rct_ps)

            # ---- G = sum_k RcT_k^T RcT_k ----
            g_ps = gp.tile([NR, NR], F32, space="PSUM")
            for k in range(NCH):
                rct_k = rctb[:, k * NR:(k + 1) * NR]
                nc.tensor.matmul(g_ps, lhsT=rct_k, rhs=rct_k,
                                 start=(k == 0), stop=(k == NCH - 1))

            # ---- K_alpha, one Newton step, W^T ----
            gm_f = sb.tile([NR, NR], F32)
            nc.vector.tensor_mul(gm_f, g_ps, c_amask)
            ka_f = sb.tile([NR, NR], F32)
            nc.vector.tensor_add(ka_f, gm_f, c_alI)
            ka_b = sb.tile([NR, NR], BF16)
            nc.scalar.copy(ka_b, ka_f)                     # ACT, overlaps DVE
            y1_b = sb.tile([NR, NR], BF16)
            nc.vector.tensor_sub(y1_b, c_2I, ka_f)         # Y1 = 2I - Ka
            p_ps = pipe.tile([NR, NR], F32, space="PSUM", tag="pp")
            nc.tensor.matmul(p_ps, lhsT=ka_b, rhs=y1_b, start=True, stop=True)
            qq_b = sb.tile([NR, NR], BF16)
            nc.vector.tensor_sub(qq_b, c_2I, p_ps)         # 2I - Ka Y1
            y2_ps = pipe.tile([NR, NR], F32, space="PSUM", tag="pp")
            nc.tensor.matmul(y2_ps, lhsT=y1_b, rhs=qq_b, start=True, stop=True)
            y2_b = sb.tile([NR, NR], BF16)
            nc.scalar.copy(y2_b, y2_ps)
            hy_ps = pipe.tile([NR, NR], F32, space="PSUM", tag="pp")
            nc.tensor.matmul(hy_ps, lhsT=c_alHb, rhs=y2_b, start=True,
                             stop=True)
            wt_b = sb.tile([NR, NR], BF16)
            nc.vector.tensor_sub(wt_b, c_I, hy_ps)         # W^T = I - alH Y

            # ---- C^T chunks (bf16) + squares for ||C||^2 ----
            ctb = sb.tile([128, NCH * NR], BF16)
            csqb = sb.tile([128, NCH * NR], BF16)
            for p in range(2):
                ct_ps = pipe.tile([128, 4 * NR], F32, space="PSUM", tag="pp")
                for kk in range(4):
                    k = 4 * p + kk
                    nc.tensor.matmul(ct_ps[:, kk * NR:(kk + 1) * NR],
                                     lhsT=rb[:, k * 128:(k + 1) * 128],
                                     rhs=wt_b, start=True, stop=True)
                sl = slice(p * 4 * NR, (p + 1) * 4 * NR)
                nc.vector.tensor_copy(ctb[:, sl], ct_ps)
                nc.scalar.activation(csqb[:, sl], ct_ps, AF.Square)

            # ---- cn row: [1,100] = sum_d -0.5 * C^T(d,j)^2 (fp32 result) ----
            cn_ps = cnp.tile([1, NR], F32, space="PSUM")
            for k in range(NCH):
                nc.tensor.matmul(cn_ps, lhsT=neghb,
                                 rhs=csqb[:, k * NR:(k + 1) * NR],
                                 start=(k == 0), stop=(k == NCH - 1))
            cn_f = sb.tile([1, NR], F32)
            nc.scalar.copy(cn_f, cn_ps)

            # ---- QC per task + fp32 rank-1 cn fold + fused epilogue ----
            out_sb = sb.tile([NQ, TPC * NJ], F32)
            for t in range(TPC):
                qc_ps = qcp.tile([NQ, NJ], F32, space="PSUM", tag="qc",
                                 name=f"qc{t}")
                for k in range(NCH):
                    lhs = qtb[:, k * 300 + t * NQ:k * 300 + (t + 1) * NQ]
                    rhs = ctb[:, k * NR + t * NJ:k * NR + t * NJ + NJ]
                    nc.tensor.matmul(qc_ps, lhsT=lhs, rhs=rhs,
                                     start=(k == 0), stop=False)
                nc.tensor.matmul(qc_ps, lhsT=ones75,
                                 rhs=cn_f[0:1, t * NJ:(t + 1) * NJ],
                                 start=False, stop=True)
                # logits = (2/D)*psum + (-qn/D), one dual-op DVE instr
                nc.vector.tensor_scalar(out_sb[:, t * NJ:(t + 1) * NJ],
                                        qc_ps, 2.0 / D, qbias[:, t:t + 1],
                                        ALU.mult, ALU.add)
            nc.sync.dma_start(out=out_d[:], in_=out_sb)

    nc.finalize()
    return nc


_NC_CACHE = None


def _get_nc():
    global _NC_CACHE
    if _NC_CACHE is None:
        _NC_CACHE = build_nc()
    return _NC_CACHE


def make_in_maps(query, support, noise):
    query = np.asarray(query, dtype=np.float32)
    support = np.asarray(support, dtype=np.float32)
    noise = np.asarray(noise, dtype=np.float32)
    cF, cB = _host_consts()
    in_maps = []
    for c in range(N_CORES):
        ts = slice(c * TPC, (c + 1) * TPC)
        qc = query[ts]                                   # (4, 75, 1024)
        qt = np.ascontiguousarray(
            qc.transpose(2, 0, 1).reshape(NCH, 128, TPC * NQ)
        ).astype(ml_dtypes.bfloat16)
        qn = np.ascontiguousarray(
            qc.transpose(1, 0, 2).reshape(NQ, TPC * D)
        ).astype(ml_dtypes.bfloat16)
        in_maps.append({
            "qt": qt,
            "qn": qn,
            "sn": np.ascontiguousarray(support[ts]).reshape(NR, D),
            "nz": np.ascontiguousarray(
                noise[:, ts].transpose(1, 0, 2, 3)).reshape(NR, D),
            "cF": cF,
            "cB": cB,
        })
    return in_maps


def kernel(query, support, noise, support_labels=None, n_way=None, n_shot=None,
           **_unused):
    nc = _get_nc()
    in_maps = make_in_maps(query, support, noise)
    res = run_bass_kernel_spmd(nc, in_maps, list(range(N_CORES)))
    outs = [np.asarray(r["out"]).reshape(NQ, TPC, NJ).transpose(1, 0, 2)
            for r in res.results]
    full = np.concatenate(outs, axis=0)            # (32, 75, 25)
    return full.reshape(T_FULL, NQ, NW, NS).astype(np.float32)
